# revision 4
# baseline (speedup 1.0000x reference)
"""Differential attention (B=2, N=2048, D=1024, H=8, HEAD_DIM=128) on 8 trn2
NeuronCores. Head-parallel: core h computes head h end-to-end. The heads->tokens
reshard is pipelined: after each 512-token block tb finishes its attention
epilogue, a small AllToAll exchanges that block (dest-sliced into 64-token
chunks), and the output projection for the block runs interleaved with the next
block's attention. Each core emits the token set {tb*512 + c*64 + i} (c = core).

Layout: activations feature-major ([feature, token]); host supplies x
pre-transposed in bf16 and reassembles the strided token output.
"""

import numpy as np

import concourse.bass as bass
import concourse.mybir as mybir
import concourse.tile as tile
from concourse.bass_utils import run_bass_kernel_spmd
from concourse.vector_clock import ScopedClock

# ---------------------------------------------------------------- constants
B, N, D = 2, 2048, 1024
H, HD = 8, 128
DQK = HD // 2
PROJ = H * HD
T = B * N  # 4096 flattened tokens
NCORES = 8
TBLK = T // NCORES  # 512 tokens per core for the output projection
LAMBDA_INIT = 0.8 - 0.6 * float(np.exp(-0.3 * 12))
SCALE = DQK ** -0.5
EPS = 1e-6

KB = N // 128  # 16 key chunks per batch
QB = N // 512  # 4 query blocks of 512 per batch
NBLK = B * QB  # 8 global 512-token blocks == NCORES

FP = mybir.dt.float32
FR = mybir.dt.float32r
BF = mybir.dt.bfloat16

# (stream, sub) -> column offset of the 129-wide accumulator region inside the
# 3-bank U tile [128, 1536]; regions never cross a 512-col PSUM bank boundary.
UREG = [0, 129, 258, 512, 641, 770, 1024, 1153]  # index = s * 4 + sub


# ------------------------------------------------- walrus drain workaround
# This container's walrus rejects instructions carrying >1 sync wait.
def _split_waits(nc, inst, max_waits=1):
    si = inst.ins.sync_info
    if si is None:
        return
    waits = list(si.on_wait)
    if len(waits) <= max_waits:
        return
    si.on_wait = waits[:max_waits]
    for w in waits[max_waits:]:
        d2 = nc.sync.drain(fusable=False)
        si2 = d2.ins.sync_info
        if si2 is None:
            d2.ins.sync_info = mybir.SyncInfo(on_wait=[w], on_update=[])
        else:
            si2.on_wait = [w]


def _split_all_multiwaits(nc, max_waits=1):
    """Hoist extra sync-waits onto fresh NoOps inserted just before the
    instruction on the same engine (engines dispatch in order)."""
    uid = 0
    for fn in nc.m.functions:
        for bb in fn.blocks:
            il = bb.instructions
            changed = False
            out = []
            for inst in il:
                si = inst.sync_info
                waits = list(si.on_wait) if si is not None else []
                if len(waits) > max_waits:
                    for w in waits[:-max_waits]:
                        ev = mybir.InstEventSemaphore(
                            name=f"waitsplit_{uid}",
                            sync_info=mybir.SyncInfo(on_wait=[w], on_update=[]),
                            engine=inst.engine,
                        )
                        uid += 1
                        out.append(ev)
                    si.on_wait = waits[-max_waits:]
                    if inst.sync_info is not si:
                        inst.sync_info = si
                    changed = True
                out.append(inst)
            if changed:
                bb.instructions = out


def _patched_drain_and_barrier(self, tick_clock, wait_clock):
    nc = self.nc
    drain_inst = nc.sync.drain(fusable=False)
    wait_clock.add_sem_waits(
        drain_inst.ins, ScopedClock({None: tick_clock.global_clock})
    )
    _split_waits(nc, drain_inst)
    nc.all_engine_barrier()
    assert self.sems is not None
    popped = nc._tile_sem_poison_stack.pop()
    assert popped is self._sem_poison
    nc.clear_and_free_semaphores(list(self.sems.allocated().values()))
    nc.all_engine_barrier()


tile.TileContext._drain_and_barrier = _patched_drain_and_barrier


# ---------------------------------------------------------------- program
def build_program(dbg=False, reps=1, skip_cc=False):
    nc = bass.Bass(
        "TRN2",
        target_bir_lowering=False,
        debug=False,
        enable_asserts=True,
        num_devices=NCORES,
    )

    DC = D // 128  # contraction chunks for the qkv projection
    xT = nc.dram_tensor("xT", [D, T], BF, kind="ExternalInput")
    wq = nc.dram_tensor("wq", [128, DC * HD], BF, kind="ExternalInput")
    wk = nc.dram_tensor("wk", [128, DC * HD], BF, kind="ExternalInput")
    wv = nc.dram_tensor("wv", [128, DC * HD], BF, kind="ExternalInput")
    wp = nc.dram_tensor("wp", [128, H * D], BF, kind="ExternalInput")
    lam = nc.dram_tensor("lam", [128, 1], FP, kind="ExternalInput")
    yT = nc.dram_tensor("yT", [D, TBLK], FP, kind="ExternalOutput")

    with tile.TileContext(nc, num_cores=NCORES) as tc:
        with (
            tc.tile_pool(name="consts", bufs=1) as consts,
            tc.tile_pool(name="dram", bufs=1, space="DRAM") as dram,
        ):
            lam_sb = consts.tile([128, 1], FP)
            nc.sync.dma_start(lam_sb[:], lam[:])

            wq_sb = consts.tile([128, DC, HD], BF)
            wk_sb = consts.tile([128, DC, HD], BF)
            wv_sb = consts.tile([128, DC, HD], BF)
            for w_dram, w_sb in ((wq, wq_sb), (wk, wk_sb), (wv, wv_sb)):
                nc.sync.dma_start(w_sb[:], w_dram.rearrange("p (c m) -> p c m", c=DC))
            wp_sb = consts.tile([128, H, D], BF)
            nc.sync.dma_start(wp_sb[:], wp.rearrange("p (h m) -> p h m", h=H))

            qT_b = [consts.tile([128, N], FR, name=f"qT_{b}") for b in range(B)]
            kT_b = [consts.tile([128, N], FR, name=f"kT_{b}") for b in range(B)]
            # v per (batch, key-chunk): [key, head_dim] plus a ones column
            # (col 128) so the PV matmul also accumulates the softmax denom.
            va_b = [consts.tile([128, KB, HD + 1], BF, name=f"va_{b}") for b in range(B)]
            for b in range(B):
                nc.vector.memset(va_b[b][:, :, HD : HD + 1], 1.0)

            # pipelined reshard buffers: [block, dest-core, feature, 64 tokens]
            a2a_in = dram.tile([NBLK, NCORES, 128, 64], BF)
            a2a_out = dram.tile([NBLK, NCORES, 128, 64], BF)

            for rep in range(reps):
                # ---------------- phase A: qkv projection (feature-major) ----
                with (
                    tc.tile_pool(name="xa", bufs=2) as xa,
                    tc.tile_pool(name="pa", bufs=2, space="PSUM") as pa,
                ):
                    xT_view = xT.rearrange("(c p) t -> p c t", p=128)
                    for tp in range(T // 1024):  # 1024-token pairs
                        b = tp // 2
                        ts2 = slice(tp * 1024, (tp + 1) * 1024)
                        xx = [
                            xa.tile([128, 1024], BF, tag=f"xx{c}", name=f"xx_{tp}_{c}")
                            for c in range(DC)
                        ]
                        for c in range(DC):
                            nc.sync.dma_start(xx[c][:], xT_view[:, c, ts2])
                        for half in range(2):
                            tb = tp * 2 + half
                            hs = slice(half * 512, (half + 1) * 512)
                            bs = slice((tb % QB) * 512, (tb % QB + 1) * 512)
                            qps = pa.tile([128, 512], FP, tag="qps", name=f"qps_{tb}")
                            kps = pa.tile([128, 512], FP, tag="kps", name=f"kps_{tb}")
                            for c in range(DC):
                                f = dict(start=(c == 0), stop=(c == DC - 1))
                                nc.tensor.matmul(qps[:], wq_sb[:, c, :], xx[c][:, hs], **f)
                                nc.tensor.matmul(kps[:], wk_sb[:, c, :], xx[c][:, hs], **f)
                            # v directly token-major: out[tok, hd] from
                            # stationary x chunks — no transposes needed.
                            vps = pa.tile([128, 4, HD], FP, tag="vps", name=f"vps_{tb}")
                            for sub in range(4):
                                ss = slice(half * 512 + sub * 128,
                                           half * 512 + (sub + 1) * 128)
                                for c in range(DC):
                                    nc.tensor.matmul(
                                        vps[:, sub, :],
                                        xx[c][:, ss],
                                        wv_sb[:, c, :],
                                        start=(c == 0 and sub == 0),
                                        stop=(c == DC - 1),
                                    )
                            nc.vector.tensor_copy(qT_b[b][:, bs], qps[:])
                            nc.vector.tensor_copy(kT_b[b][:, bs], kps[:])
                            kb0 = (tb % QB) * 4
                            nc.vector.tensor_copy(
                                va_b[b][:, kb0 : kb0 + 4, 0:HD], vps[:]
                            )

                # ---------------- phase B: differential attention ------------
                # PSUM budget (8 banks): s12 2 tiles x 2 banks, U 3 banks,
                # yps (output projection) 1 bank.
                with (
                    tc.tile_pool(name="ps", bufs=1, space="PSUM") as ps,
                    tc.tile_pool(name="pu", bufs=1, space="PSUM") as pu,
                    tc.tile_pool(name="pc", bufs=1, space="PSUM") as pc,
                    tc.tile_pool(name="pp", bufs=4) as pp,
                    tc.tile_pool(name="se", bufs=2) as se,
                    tc.tile_pool(name="so", bufs=4) as so,
                    tc.tile_pool(name="sc", bufs=2) as sc,
                ):
                    def emit_cproj(t):
                        # output projection for block t (all heads arrived via
                        # collective t): 512 strided tokens for this core.
                        aa = sc.tile([128, H, 64], BF, tag="aa", name=f"aa_{rep}_{t}")
                        for hh in range(H):
                            nc.sync.dma_start(aa[:, hh, :], a2a_out[t, hh])
                        yps = pc.tile([128, 8, 64], FP, tag="yps", name=f"yps_{rep}_{t}")
                        for oc in range(8):
                            for hh in range(H):
                                nc.tensor.matmul(
                                    yps[:, oc, :],
                                    wp_sb[:, hh, oc * 128 : (oc + 1) * 128],
                                    aa[:, hh, :],
                                    start=(oc == 0 and hh == 0),
                                    stop=(hh == H - 1),
                                )
                        yo = sc.tile([128, 8, 64], FP, tag="yo", name=f"yo_{rep}_{t}")
                        nc.vector.tensor_copy(yo[:], yps[:])
                        nc.sync.dma_start(
                            yT.rearrange("(oc p) t -> p oc t", p=128)[
                                :, :, t * 64 : (t + 1) * 64
                            ],
                            yo[:],
                        )

                    for b in range(B):
                        for qb in range(QB):
                            tb = b * QB + qb  # global 512-token block id
                            qs = slice(qb * 512, (qb + 1) * 512)
                            U = pu.tile([128, 1536], FP, tag="U", bufs=1,
                                        name=f"U_{rep}_{tb}")
                            for kb in range(KB):
                                ks = slice(kb * 128, (kb + 1) * 128)
                                s12 = ps.tile([128, 1024], FP, tag="s12", bufs=2)
                                # S^T tiles [key, query] for both q/k streams.
                                nc.tensor.matmul(
                                    s12[:, 0:512],
                                    kT_b[b][0:64, ks],
                                    qT_b[b][0:64, qs],
                                    start=True, stop=True,
                                )
                                nc.tensor.matmul(
                                    s12[:, 512:1024],
                                    kT_b[b][64:128, ks],
                                    qT_b[b][64:128, qs],
                                    start=True, stop=True,
                                )
                                p12 = pp.tile([128, 1024], BF, tag="p12")
                                nc.scalar.activation(
                                    p12[:], s12[:], mybir.ActivationFunctionType.Exp
                                )
                                vak = va_b[b][:, kb, :]
                                for s in range(2):
                                    for sub in range(4):
                                        # start=True clears has_written for the
                                        # whole PSUM bank; only the first
                                        # matmul touching each U bank sets it.
                                        r = UREG[s * 4 + sub]
                                        # bank boundaries at region indices
                                        # 0 (col 0), 3 (col 512), 6 (col 1024)
                                        first_of_bank = (
                                            kb == 0 and (s, sub) in
                                            ((0, 0), (0, 3), (1, 2))
                                        )
                                        nc.tensor.matmul(
                                            U[:, r : r + HD + 1],
                                            p12[:, s * 512 + sub * 128 : s * 512 + (sub + 1) * 128],
                                            vak,
                                            start=first_of_bank,
                                            stop=(kb == KB - 1),
                                        )
                                # interleave the previous block's output
                                # projection mid-block so PE fills ACT-bound
                                # gaps (collective tb-1 has had time to land)
                                if kb == 7 and tb >= 1 and not skip_cc:
                                    emit_cproj(tb - 1)
                            # epilogue: softmax normalize, differential
                            # combine, RMSNorm; emit transposed 64-token
                            # chunks into the per-block A2A buffer.
                            Usb = se.tile([128, 1536], FP, tag="usb",
                                          name=f"usb_{rep}_{tb}")
                            nc.vector.tensor_copy(Usb[:], U[:])
                            ms = se.tile([128, 4], FP, tag="ms")
                            od_t = [None] * 4
                            for sub in range(4):
                                r1c = UREG[sub] + HD
                                r2c = UREG[4 + sub] + HD
                                r1 = se.tile([128, 1], FP, tag=f"r1_{sub}")
                                r2 = se.tile([128, 1], FP, tag=f"r2_{sub}")
                                nc.vector.reciprocal(r1[:], Usb[:, r1c : r1c + 1])
                                nc.vector.reciprocal(r2[:], Usb[:, r2c : r2c + 1])
                                r2l = se.tile([128, 1], FP, tag=f"r2l_{sub}")
                                nc.vector.tensor_mul(r2l[:], r2[:], lam_sb[:])
                                t1 = se.tile([128, 128], FP, tag=f"t1_{sub}")
                                t2 = se.tile([128, 128], FP, tag=f"t2_{sub}")
                                nc.vector.tensor_scalar_mul(
                                    t1[:], Usb[:, UREG[sub] : UREG[sub] + HD], r1[:]
                                )
                                nc.vector.tensor_scalar_mul(
                                    t2[:], Usb[:, UREG[4 + sub] : UREG[4 + sub] + HD],
                                    r2l[:],
                                )
                                od = se.tile([128, 128], FP, tag=f"od_{sub}")
                                nc.vector.tensor_sub(od[:], t1[:], t2[:])
                                od_t[sub] = od
                                sq = se.tile([128, 128], FP, tag=f"sq_{sub}")
                                nc.vector.tensor_mul(sq[:], od[:], od[:])
                                ssum = se.tile([128, 1], FP, tag=f"ss_{sub}")
                                nc.vector.tensor_reduce(
                                    ssum[:], sq[:], mybir.AxisListType.X,
                                    mybir.AluOpType.add,
                                )
                                nc.vector.tensor_scalar(
                                    ms[:, sub : sub + 1], ssum[:], 1.0 / HD, EPS,
                                    mybir.AluOpType.mult, mybir.AluOpType.add,
                                )
                            # rsqrt(ms) = exp(-0.5*ln(ms)) batched over the 4
                            # subs; Log/Exp share one ACT table set with the
                            # attention exps (no table-switch thrash).
                            rt = se.tile([128, 4], FP, tag="rt")
                            nc.scalar.activation(
                                rt[:], ms[:], mybir.ActivationFunctionType.Ln
                            )
                            rs = se.tile([128, 4], FP, tag="rs")
                            nc.scalar.activation(
                                rs[:], rt[:], mybir.ActivationFunctionType.Exp,
                                scale=-0.5,
                            )
                            for sub in range(4):
                                on = se.tile([128, 128], BF, tag=f"on_{sub}")
                                nc.vector.tensor_scalar_mul(
                                    on[:], od_t[sub][:], rs[:, sub : sub + 1]
                                )
                                # transpose via the DMA xbar so the PE queue is
                                # never blocked on the epilogue
                                onT = so.tile([128, 128], BF, tag="onT")
                                nc.sync.dma_start_transpose(onT[:], on[:])
                                nc.sync.dma_start(
                                    a2a_in[tb, 2 * sub], onT[:, 0:64]
                                )
                                nc.sync.dma_start(
                                    a2a_in[tb, 2 * sub + 1], onT[:, 64:128]
                                )
                            # per-block collective: dest-sliced exchange so
                            # the reshard pipelines with later blocks
                            if skip_cc:
                                nc.sync.dma_start(a2a_out[tb], a2a_in[tb])
                            else:
                                nc.gpsimd.collective_compute(
                                    "AllToAll",
                                    mybir.AluOpType.bypass,
                                    replica_groups=[list(range(NCORES))],
                                    ins=[a2a_in[tb].opt()],
                                    outs=[a2a_out[tb].opt()],
                                )
                    if skip_cc:
                        for t in range(NBLK):
                            emit_cproj(t)
                    else:
                        emit_cproj(NBLK - 1)

    _split_all_multiwaits(nc)
    return nc


_PROGRAM = None


def _get_program():
    global _PROGRAM
    if _PROGRAM is None:
        _PROGRAM = build_program()
    return _PROGRAM


# ---------------------------------------------------------------- host side
def _prep_in_maps(x, w_qkv, w_proj, lambda_q1, lambda_k1, lambda_q2, lambda_k2,
                  rms_weight):
    import ml_dtypes

    x = np.asarray(x, dtype=np.float32)
    w_qkv = np.asarray(w_qkv, dtype=np.float32)
    w_proj = np.asarray(w_proj, dtype=np.float32)
    xT = np.ascontiguousarray(x.reshape(T, D).T).astype(ml_dtypes.bfloat16)
    lam_val = (
        float(np.exp(np.sum(np.asarray(lambda_q1, np.float64) * np.asarray(lambda_k1, np.float64))))
        - float(np.exp(np.sum(np.asarray(lambda_q2, np.float64) * np.asarray(lambda_k2, np.float64))))
        + LAMBDA_INIT
    )
    lam_arr = np.full((128, 1), lam_val, dtype=np.float32)
    # fold rms_weight and (1 - lambda_init) into the output projection rows
    rw = np.asarray(rms_weight, np.float32)
    wp_full = np.ascontiguousarray(
        w_proj * np.tile(rw, H)[:, None] * np.float32(1.0 - LAMBDA_INIT)
    )

    # device-friendly layouts: weights arranged so each DMA descriptor is a
    # long contiguous run per partition
    def chunked(w):  # [D, HD] -> [128, DC*HD] with [p, c*HD+m] = w[c*128+p, m]
        dc = D // 128
        return np.ascontiguousarray(
            w.reshape(dc, 128, HD).transpose(1, 0, 2).reshape(128, dc * HD)
        ).astype(ml_dtypes.bfloat16)

    wp_dev = np.ascontiguousarray(
        wp_full.reshape(H, 128, D).transpose(1, 0, 2).reshape(128, H * D)
    ).astype(ml_dtypes.bfloat16)
    in_maps = []
    for h in range(NCORES):
        hs = slice(h * HD, (h + 1) * HD)
        in_maps.append(
            {
                "xT": xT,
                "wq": chunked(np.ascontiguousarray(w_qkv[:, hs]) * np.float32(SCALE)),
                "wk": chunked(w_qkv[:, PROJ + h * HD : PROJ + (h + 1) * HD]),
                "wv": chunked(w_qkv[:, 2 * PROJ + h * HD : 2 * PROJ + (h + 1) * HD]),
                "wp": wp_dev,
                "lam": lam_arr,
            }
        )
    return in_maps


def _assemble(results):
    y = np.empty((T, D), dtype=np.float32)
    for c in range(NCORES):
        yTc = results[c]["yT"]  # [D, 512], cols ordered (tb, i)
        for tb in range(NBLK):
            y[tb * 512 + c * 64 : tb * 512 + (c + 1) * 64, :] = (
                yTc[:, tb * 64 : (tb + 1) * 64].T
            )
    return y.reshape(B, N, D)


def kernel(x, w_qkv, w_proj, lambda_q1, lambda_k1, lambda_q2, lambda_k2,
           rms_weight):
    nc = _get_program()
    in_maps = _prep_in_maps(
        x, w_qkv, w_proj, lambda_q1, lambda_k1, lambda_q2, lambda_k2, rms_weight
    )
    res = run_bass_kernel_spmd(nc, in_maps, list(range(NCORES)))
    return _assemble(res.results)


# revision 13
# speedup vs baseline: 1.3434x; 1.3434x over previous
"""Differential attention (B=2, N=2048, D=1024, H=8, HEAD_DIM=128) on 8 trn2
NeuronCores. Head-parallel: core h computes head h end-to-end. The heads->tokens
reshard is pipelined: after each 512-token block tb finishes its attention
epilogue, a small AllToAll exchanges that block (dest-sliced into 64-token
chunks), and the output projection for the block runs interleaved with later
blocks' attention. Each core emits the token set {tb*512 + c*64 + i} (c = core).

Schedule: qkv for batch 0, then batch-0 attention with batch-1 qkv matmuls
interleaved into the PE slack of the ACT(exp)-bound loop (keeps the PE
continuously busy so HAM holds the 2.4GHz pstate), then batch-1 attention with
the output projections interleaved. A dummy collective at program start absorbs
the ~20us CC cold-start off the critical path; a dummy exp preloads the ACT
table set.
"""

import numpy as np

import concourse.bass as bass
import concourse.mybir as mybir
import concourse.tile as tile
from concourse.bass_utils import run_bass_kernel_spmd
from concourse.vector_clock import ScopedClock

# ---------------------------------------------------------------- constants
B, N, D = 2, 2048, 1024
H, HD = 8, 128
DQK = HD // 2
PROJ = H * HD
T = B * N  # 4096 flattened tokens
NCORES = 8
TBLK = T // NCORES  # 512 tokens per core for the output projection
LAMBDA_INIT = 0.8 - 0.6 * float(np.exp(-0.3 * 12))
SCALE = DQK ** -0.5
EPS = 1e-6

KB = N // 128  # 16 key chunks per batch
QB = N // 512  # 4 query blocks of 512 per batch
NBLK = B * QB  # 8 global 512-token blocks == NCORES
DC = D // 128  # contraction chunks for the qkv projection

FP = mybir.dt.float32
FR = mybir.dt.float32r
BF = mybir.dt.bfloat16

# (stream, sub) -> column offset of the 129-wide accumulator region inside the
# 3-bank U tile [128, 1536]; regions never cross a 512-col PSUM bank boundary.
UREG = [0, 129, 258, 512, 641, 770, 1024, 1153]  # index = s * 4 + sub
UFIRST = ((0, 0), (0, 3), (1, 2))  # (s, sub) pairs that first touch a U bank


# ------------------------------------------------- walrus drain workaround
# This container's walrus rejects instructions carrying >1 sync wait.
def _split_waits(nc, inst, max_waits=1):
    si = inst.ins.sync_info
    if si is None:
        return
    waits = list(si.on_wait)
    if len(waits) <= max_waits:
        return
    si.on_wait = waits[:max_waits]
    for w in waits[max_waits:]:
        d2 = nc.sync.drain(fusable=False)
        si2 = d2.ins.sync_info
        if si2 is None:
            d2.ins.sync_info = mybir.SyncInfo(on_wait=[w], on_update=[])
        else:
            si2.on_wait = [w]


def _split_all_multiwaits(nc, max_waits=1):
    """Hoist extra sync-waits onto fresh NoOps inserted just before the
    instruction on the same engine (engines dispatch in order)."""
    uid = 0
    for fn in nc.m.functions:
        for bb in fn.blocks:
            il = bb.instructions
            changed = False
            out = []
            for inst in il:
                si = inst.sync_info
                waits = list(si.on_wait) if si is not None else []
                if len(waits) > max_waits:
                    for w in waits[:-max_waits]:
                        ev = mybir.InstEventSemaphore(
                            name=f"waitsplit_{uid}",
                            sync_info=mybir.SyncInfo(on_wait=[w], on_update=[]),
                            engine=inst.engine,
                        )
                        uid += 1
                        out.append(ev)
                    si.on_wait = waits[-max_waits:]
                    if inst.sync_info is not si:
                        inst.sync_info = si
                    changed = True
                out.append(inst)
            if changed:
                bb.instructions = out


def _patched_drain_and_barrier(self, tick_clock, wait_clock):
    nc = self.nc
    drain_inst = nc.sync.drain(fusable=False)
    wait_clock.add_sem_waits(
        drain_inst.ins, ScopedClock({None: tick_clock.global_clock})
    )
    _split_waits(nc, drain_inst)
    nc.all_engine_barrier()
    assert self.sems is not None
    popped = nc._tile_sem_poison_stack.pop()
    assert popped is self._sem_poison
    nc.clear_and_free_semaphores(list(self.sems.allocated().values()))
    nc.all_engine_barrier()


tile.TileContext._drain_and_barrier = _patched_drain_and_barrier


# ---------------------------------------------------------------- program
def build_program(dbg=False, reps=1, skip_cc=False):
    nc = bass.Bass(
        "TRN2",
        target_bir_lowering=False,
        debug=False,
        enable_asserts=True,
        num_devices=NCORES,
    )

    xT = nc.dram_tensor("xT", [D, T], BF, kind="ExternalInput")
    wq = nc.dram_tensor("wq", [128, DC * HD], BF, kind="ExternalInput")
    wk = nc.dram_tensor("wk", [128, DC * HD], BF, kind="ExternalInput")
    wv = nc.dram_tensor("wv", [128, DC * HD], BF, kind="ExternalInput")
    wp = nc.dram_tensor("wp", [128, H * D], BF, kind="ExternalInput")
    lam = nc.dram_tensor("lam", [128, 1], FP, kind="ExternalInput")
    yT = nc.dram_tensor("yT", [D, TBLK], FP, kind="ExternalOutput")

    with tile.TileContext(nc, num_cores=NCORES) as tc:
        with (
            tc.tile_pool(name="consts", bufs=1) as consts,
            tc.tile_pool(name="xa", bufs=2) as xa,
            tc.tile_pool(name="pp", bufs=4) as pp,
            tc.tile_pool(name="se", bufs=2) as se,
            tc.tile_pool(name="so", bufs=4) as so,
            tc.tile_pool(name="sc", bufs=2) as sc,
            tc.tile_pool(name="dram", bufs=1, space="DRAM") as dram,
        ):
            lam_sb = consts.tile([128, 1], FP)
            nc.sync.dma_start(lam_sb[:], lam[:])
            wq_sb = consts.tile([128, DC, HD], BF)
            wk_sb = consts.tile([128, DC, HD], BF)
            wv_sb = consts.tile([128, DC, HD], BF)
            nc.sync.dma_start(wq_sb[:], wq.rearrange("p (c m) -> p c m", c=DC))

            # warm the ACT table set (exp/ln) off the critical path
            warm = consts.tile([128, 1], FP)
            nc.scalar.activation(warm[:], lam_sb[:],
                                 mybir.ActivationFunctionType.Exp)

            # warm the collective stack: tiny AllToAll absorbs CC init
            cc_warm_in = dram.tile([NCORES, 1], BF)
            cc_warm_out = dram.tile([NCORES, 1], BF)
            warm8 = consts.tile([8, 1], BF)
            nc.vector.memset(warm8[:], 0.0)
            eps_sb = consts.tile([128, 1], FP)
            nc.vector.memset(eps_sb[:], EPS)
            nc.sync.dma_start(cc_warm_in[:], warm8[:])
            if not skip_cc:
                nc.gpsimd.collective_compute(
                    "AllToAll",
                    mybir.AluOpType.bypass,
                    replica_groups=[list(range(NCORES))],
                    ins=[cc_warm_in.opt()],
                    outs=[cc_warm_out.opt()],
                )

            wp_sb = consts.tile([128, H, D], BF)

            qT_b = [consts.tile([128, N], FR, name=f"qT_{b}") for b in range(B)]
            kT_b = [consts.tile([128, N], FR, name=f"kT_{b}") for b in range(B)]
            # v per (batch, key-chunk): [key, head_dim] plus a ones column
            # (col 128) so the PV matmul also accumulates the softmax denom.
            va_b = [consts.tile([128, KB, HD + 1], BF, name=f"va_{b}") for b in range(B)]
            for b in range(B):
                nc.vector.memset(va_b[b][:, :, HD : HD + 1], 1.0)

            # pipelined reshard buffers, paired blocks per collective:
            # [pair, dest-core, block-within-pair, feature, 64 tokens]
            a2a_in = dram.tile([NBLK // 2, NCORES, 2, 128, 64], BF)
            a2a_out = dram.tile([NBLK // 2, NCORES, 2, 128, 64], BF)

            xT_view = xT.rearrange("(c p) t -> p c t", p=128)

            def load_x(tp, engines):
                """DMA the 8 contraction chunks for 1024-token pair tp."""
                ts2 = slice(tp * 1024, (tp + 1) * 1024)
                xx = []
                for c in range(DC):
                    t = xa.tile([128, 1024], BF, tag=f"xx{c}", name=f"xx_{tp}_{c}")
                    engines[c % len(engines)].dma_start(t[:], xT_view[:, c, ts2])
                    xx.append(t)
                return xx

            def a_group(xx, tp, half, which, psum_tile):
                """Emit one qkv matmul group (q/k/v) for a 512-token half and
                copy the result out to the feature-major SBUF activations."""
                b = tp // 2
                tb = tp * 2 + half
                hs = slice(half * 512, (half + 1) * 512)
                bs = slice((tb % QB) * 512, (tb % QB + 1) * 512)
                if which == 0:
                    for c in range(DC):
                        nc.tensor.matmul(psum_tile[:], wq_sb[:, c, :], xx[c][:, hs],
                                         start=(c == 0), stop=(c == DC - 1))
                    nc.vector.tensor_copy(qT_b[b][:, bs], psum_tile[:])
                elif which == 1:
                    for c in range(DC):
                        nc.tensor.matmul(psum_tile[:], wk_sb[:, c, :], xx[c][:, hs],
                                         start=(c == 0), stop=(c == DC - 1))
                    nc.vector.tensor_copy(kT_b[b][:, bs], psum_tile[:])
                else:
                    # v directly token-major: out[tok, hd] from stationary x
                    vv = psum_tile[:].rearrange("p (s m) -> p s m", s=4)
                    for sub in range(4):
                        ss = slice(half * 512 + sub * 128,
                                   half * 512 + (sub + 1) * 128)
                        for c in range(DC):
                            nc.tensor.matmul(
                                vv[:, sub, :], xx[c][:, ss], wv_sb[:, c, :],
                                start=(c == 0 and sub == 0),
                                stop=(c == DC - 1),
                            )
                    kb0 = (tb % QB) * 4
                    nc.vector.tensor_copy(va_b[b][:, kb0 : kb0 + 4, 0:HD], vv[:])

            def score_exp(ps_pool, b, qb, kb):
                """scores + exp for one 128-key chunk of block (b, qb)."""
                qs = slice(qb * 512, (qb + 1) * 512)
                ks = slice(kb * 128, (kb + 1) * 128)
                s12 = ps_pool.tile([128, 1024], FP, tag="s12", bufs=2)
                nc.tensor.matmul(s12[:, 0:512], kT_b[b][0:64, ks],
                                 qT_b[b][0:64, qs], start=True, stop=True)
                nc.tensor.matmul(s12[:, 512:1024], kT_b[b][64:128, ks],
                                 qT_b[b][64:128, qs], start=True, stop=True)
                p12 = pp.tile([128, 1024], BF, tag="p12")
                nc.scalar.activation(p12[:], s12[:],
                                     mybir.ActivationFunctionType.Exp)
                return p12

            def pv(b, p12, U, kb):
                vak = va_b[b][:, kb, :]
                for s in range(2):
                    for sub in range(4):
                        r = UREG[s * 4 + sub]
                        nc.tensor.matmul(
                            U[:, r : r + HD + 1],
                            p12[:, s * 512 + sub * 128 : s * 512 + (sub + 1) * 128],
                            vak,
                            start=(kb == 0 and (s, sub) in UFIRST),
                            stop=(kb == KB - 1),
                        )

            def epilogue_part1(tb, U):
                """softmax normalize + differential combine + sum of squares.
                Returns (od tiles, ssum4) consumed by the deferred part 2."""
                Usb = se.tile([128, 1536], FP, tag="usb", name=f"usb_{tb}")
                nc.vector.tensor_copy(Usb[:], U[:])
                ssum4 = se.tile([128, 4], FP, tag="ms")
                od_t = [None] * 4
                for sub in range(4):
                    r1c = UREG[sub] + HD
                    r2c = UREG[4 + sub] + HD
                    r1 = se.tile([128, 1], FP, tag=f"r1_{sub}")
                    r2 = se.tile([128, 1], FP, tag=f"r2_{sub}")
                    nc.vector.reciprocal(r1[:], Usb[:, r1c : r1c + 1])
                    nc.vector.reciprocal(r2[:], Usb[:, r2c : r2c + 1])
                    r2l = se.tile([128, 1], FP, tag=f"r2l_{sub}")
                    nc.vector.tensor_mul(r2l[:], r2[:], lam_sb[:])
                    t1 = se.tile([128, 128], FP, tag=f"t1_{sub}")
                    t2 = se.tile([128, 128], FP, tag=f"t2_{sub}")
                    nc.vector.tensor_scalar_mul(
                        t1[:], Usb[:, UREG[sub] : UREG[sub] + HD], r1[:]
                    )
                    nc.vector.tensor_scalar_mul(
                        t2[:], Usb[:, UREG[4 + sub] : UREG[4 + sub] + HD], r2l[:]
                    )
                    od = se.tile([128, 128], FP, tag=f"od_{sub}")
                    nc.vector.tensor_sub(od[:], t1[:], t2[:])
                    od_t[sub] = od
                    sq = se.tile([128, 128], FP, tag=f"sq_{sub}")
                    nc.vector.tensor_mul(sq[:], od[:], od[:])
                    nc.vector.tensor_reduce(
                        ssum4[:, sub : sub + 1], sq[:], mybir.AxisListType.X,
                        mybir.AluOpType.add,
                    )
                return od_t, ssum4

            def epilogue_part2(tb, od_t, ssum4):
                """RMSNorm scale (Ln/Exp share the attention exps' ACT table
                set; 1/HD and EPS folded into the Ln's affine), transpose,
                stage into the A2A buffer, fire the pair collective on odd
                blocks. Deferred into the next block's chunk loop so the ACT
                queue never stalls on the DVE chain."""
                rt = se.tile([128, 4], FP, tag="rt")
                nc.scalar.activation(rt[:], ssum4[:],
                                     mybir.ActivationFunctionType.Ln,
                                     scale=1.0 / HD, bias=eps_sb[:])
                rs = se.tile([128, 4], FP, tag="rs")
                nc.scalar.activation(rs[:], rt[:],
                                     mybir.ActivationFunctionType.Exp, scale=-0.5)
                for sub in range(4):
                    on = se.tile([128, 128], BF, tag=f"on_{sub}")
                    nc.vector.tensor_scalar_mul(
                        on[:], od_t[sub][:], rs[:, sub : sub + 1]
                    )
                    onT = so.tile([128, 128], BF, tag="onT")
                    nc.sync.dma_start_transpose(onT[:], on[:])
                    nc.sync.dma_start(a2a_in[tb // 2, 2 * sub, tb % 2],
                                      onT[:, 0:64])
                    nc.sync.dma_start(a2a_in[tb // 2, 2 * sub + 1, tb % 2],
                                      onT[:, 64:128])
                if tb % 2 == 1:
                    pair = tb // 2
                    if skip_cc:
                        nc.sync.dma_start(a2a_out[pair], a2a_in[pair])
                    else:
                        nc.gpsimd.collective_compute(
                            "AllToAll",
                            mybir.AluOpType.bypass,
                            replica_groups=[list(range(NCORES))],
                            ins=[a2a_in[pair].opt()],
                            outs=[a2a_out[pair].opt()],
                        )

            def cproj(pc_pool, t):
                """output projection for block t (all heads arrived via pair
                collective t//2): this core's 64 tokens out of the block."""
                aa = sc.tile([128, H, 64], BF, tag="aa", name=f"aa_{t}")
                for hh in range(H):
                    nc.sync.dma_start(aa[:, hh, :], a2a_out[t // 2, hh, t % 2])
                yps = pc_pool.tile([128, 8, 64], FP, tag="yps", name=f"yps_{t}")
                for oc in range(8):
                    for hh in range(H):
                        nc.tensor.matmul(
                            yps[:, oc, :],
                            wp_sb[:, hh, oc * 128 : (oc + 1) * 128],
                            aa[:, hh, :],
                            start=(oc == 0 and hh == 0),
                            stop=(hh == H - 1),
                        )
                yo = sc.tile([128, 8, 64], FP, tag="yo", name=f"yo_{t}")
                nc.vector.tensor_copy(yo[:], yps[:])
                nc.sync.dma_start(
                    yT.rearrange("(oc p) t -> p oc t", p=128)[
                        :, :, t * 64 : (t + 1) * 64
                    ],
                    yo[:],
                )

            def b_block(ps_pool, pu_pool, b, qb, tb, fillers, pending):
                """One 512-token attention block, software-pipelined: scores
                and exp lead the PV accumulation by one chunk; the previous
                block's deferred epilogue tail lands at kb==5; `fillers`
                maps kb -> callable (batch-1 qkv groups or output proj)."""
                U = pu_pool.tile([128, 1536], FP, tag="U", name=f"U_{tb}")
                p12_prev = None
                for kb in range(KB + 1):
                    p12 = score_exp(ps_pool, b, qb, kb) if kb < KB else None
                    if kb >= 1:
                        pv(b, p12_prev, U, kb - 1)
                    if p12 is not None:
                        p12_prev = p12
                    if kb == 5 and pending is not None:
                        epilogue_part2(*pending)
                        pending = None
                    f = fillers.get(kb)
                    if f is not None:
                        f()
                od_t, ssum4 = epilogue_part1(tb, U)
                return (tb, od_t, ssum4)

            # ============ phase 1: qkv projection for batch 0 ============
            with tc.tile_pool(name="pa", bufs=2, space="PSUM") as pa:
                xx0 = load_x(0, [nc.sync, nc.scalar])
                nc.scalar.dma_start(wk_sb[:],
                                    wk.rearrange("p (c m) -> p c m", c=DC))
                nc.scalar.dma_start(wv_sb[:],
                                    wv.rearrange("p (c m) -> p c m", c=DC))
                for tp in (0, 1):
                    xx = xx0 if tp == 0 else load_x(1, [nc.sync, nc.scalar])
                    if tp == 1:
                        nc.scalar.dma_start(
                            wp_sb[:], wp.rearrange("p (h m) -> p h m", h=H))
                    for half in range(2):
                        tbh = tp * 2 + half
                        qps = pa.tile([128, 512], FP, tag="qps", name=f"q_{tbh}")
                        a_group(xx, tp, half, 0, qps)
                        kps = pa.tile([128, 512], FP, tag="kps", name=f"k_{tbh}")
                        a_group(xx, tp, half, 1, kps)
                        vps = pa.tile([128, 512], FP, tag="vps", name=f"v_{tbh}")
                        a_group(xx, tp, half, 2, vps)

            pending = None
            # ============ phase 2: batch-0 attention + batch-1 qkv =======
            # PSUM: s12 2x2 banks + U 3 banks + 1 bank rotating for the
            # batch-1 qkv groups (B-chunks between groups hide copy waits).
            with (
                tc.tile_pool(name="ps", bufs=1, space="PSUM") as ps,
                tc.tile_pool(name="pu", bufs=1, space="PSUM") as pu,
                tc.tile_pool(name="pa2", bufs=1, space="PSUM") as pa2,
            ):
                xx2 = load_x(2, [nc.sync])
                xx3 = load_x(3, [nc.sync])
                a_units = [(xx2, 2, 0), (xx2, 2, 1), (xx3, 3, 0), (xx3, 3, 1)]
                a_queue = [(xx, tp, half, which)
                           for (xx, tp, half) in a_units for which in range(3)]

                def make_afiller(i):
                    def f():
                        xx, tp, half, which = a_queue[i]
                        apt = pa2.tile([128, 512], FP, tag="aps",
                                       name=f"aps_{i}")
                        a_group(xx, tp, half, which, apt)
                    return f

                for qb in range(QB):
                    fillers = {}
                    for j, kb in enumerate((2, 7, 12)):
                        ai = qb * 3 + j
                        if ai < len(a_queue):
                            fillers[kb] = make_afiller(ai)
                    pending = b_block(ps, pu, 0, qb, qb, fillers, pending)

            # ============ phase 3: batch-1 attention + output proj =======
            with (
                tc.tile_pool(name="ps3", bufs=1, space="PSUM") as ps3,
                tc.tile_pool(name="pu3", bufs=1, space="PSUM") as pu3,
                tc.tile_pool(name="pc", bufs=1, space="PSUM") as pc,
            ):
                # pair collective p fires at kb==5 of block 2p+2; C(t) is
                # scheduled >= one block after its pair collective
                cfill = {
                    3: {6: [0], 13: [1]},
                    5: {6: [2], 13: [3]},
                    7: {6: [4], 13: [5]},
                }
                for qb in range(QB):
                    tb = QB + qb
                    fillers = {
                        kb: (lambda ts=ts: [cproj(pc, t) for t in ts])
                        for kb, ts in cfill.get(tb, {}).items()
                    }
                    pending = b_block(ps3, pu3, 1, qb, tb, fillers, pending)
                epilogue_part2(*pending)
                pending = None
                cproj(pc, 6)
                cproj(pc, 7)

    _split_all_multiwaits(nc)
    return nc


_PROGRAM = None


def _get_program():
    global _PROGRAM
    if _PROGRAM is None:
        _PROGRAM = build_program()
    return _PROGRAM


# ---------------------------------------------------------------- host side
def _prep_in_maps(x, w_qkv, w_proj, lambda_q1, lambda_k1, lambda_q2, lambda_k2,
                  rms_weight):
    import ml_dtypes

    x = np.asarray(x, dtype=np.float32)
    w_qkv = np.asarray(w_qkv, dtype=np.float32)
    w_proj = np.asarray(w_proj, dtype=np.float32)
    xT = np.ascontiguousarray(x.reshape(T, D).T).astype(ml_dtypes.bfloat16)
    lam_val = (
        float(np.exp(np.sum(np.asarray(lambda_q1, np.float64) * np.asarray(lambda_k1, np.float64))))
        - float(np.exp(np.sum(np.asarray(lambda_q2, np.float64) * np.asarray(lambda_k2, np.float64))))
        + LAMBDA_INIT
    )
    lam_arr = np.full((128, 1), lam_val, dtype=np.float32)
    # fold rms_weight and (1 - lambda_init) into the output projection rows
    rw = np.asarray(rms_weight, np.float32)
    wp_full = np.ascontiguousarray(
        w_proj * np.tile(rw, H)[:, None] * np.float32(1.0 - LAMBDA_INIT)
    )

    def chunked(w):  # [D, HD] -> [128, DC*HD] with [p, c*HD+m] = w[c*128+p, m]
        dc = D // 128
        return np.ascontiguousarray(
            w.reshape(dc, 128, HD).transpose(1, 0, 2).reshape(128, dc * HD)
        ).astype(ml_dtypes.bfloat16)

    wp_dev = np.ascontiguousarray(
        wp_full.reshape(H, 128, D).transpose(1, 0, 2).reshape(128, H * D)
    ).astype(ml_dtypes.bfloat16)
    in_maps = []
    for h in range(NCORES):
        hs = slice(h * HD, (h + 1) * HD)
        in_maps.append(
            {
                "xT": xT,
                "wq": chunked(np.ascontiguousarray(w_qkv[:, hs]) * np.float32(SCALE)),
                "wk": chunked(w_qkv[:, PROJ + h * HD : PROJ + (h + 1) * HD]),
                "wv": chunked(w_qkv[:, 2 * PROJ + h * HD : 2 * PROJ + (h + 1) * HD]),
                "wp": wp_dev,
                "lam": lam_arr,
            }
        )
    return in_maps


def _assemble(results):
    y = np.empty((T, D), dtype=np.float32)
    for c in range(NCORES):
        yTc = results[c]["yT"]  # [D, 512], cols ordered (tb, i)
        for tb in range(NBLK):
            y[tb * 512 + c * 64 : tb * 512 + (c + 1) * 64, :] = (
                yTc[:, tb * 64 : (tb + 1) * 64].T
            )
    return y.reshape(B, N, D)


def kernel(x, w_qkv, w_proj, lambda_q1, lambda_k1, lambda_q2, lambda_k2,
           rms_weight):
    nc = _get_program()
    in_maps = _prep_in_maps(
        x, w_qkv, w_proj, lambda_q1, lambda_k1, lambda_q2, lambda_k2, rms_weight
    )
    res = run_bass_kernel_spmd(nc, in_maps, list(range(NCORES)))
    return _assemble(res.results)
